# revision 11
# baseline (speedup 1.0000x reference)
"""Trainium2 Bass kernel for nn_AttentionBlock (sparse causal attention).

Math (per batch b, head h), with A = r_prime[b] (T x N):
    Omega_h = tril(A @ Q_h @ A^T)            (T x T)
    out[b]  = sum_h Omega_h @ (A @ E_h^T)    (T x N)

Strategy: data-parallel over batch (8 batches -> 8 NeuronCores).  Host
precomputes the input-linear maps C_h = (A Q_h)^T (N x T) and
Er_h = A E_h^T (T x N) in fp16 (pure GEMM re-layout of the inputs), so the
device only runs the quadratic-in-T part: per row-chunk I (C=128 rows),

    out_I^T = sum_h [ Er_{h,I}^T (mask o W_{h,I})  +  S_{h,I}^T C_{h,I} ]
    W_{h,I} = A_I C_{h,I}   (the diagonal C x C block, tril-masked)
    S_{h,I} = sum_{J<I} A_J^T Er_{h,J}   (N x N running state, PSUM-resident)

All operands live in SBUF for the whole rep (~50 KB/partition).  Heads are
split even/odd across SBUF partition halves so every K=64 matmul runs as a
row-tiled pair (tile_position (0,0)/(64,0), concurrent in the PE array) and
every M=64 matmul as a col-tiled pair ((0,0)/(0,64)).  The state PSUM bank
holds even heads at partitions 0-63 and odd at 64-127, which is exactly the
stacked layout the inter matmul needs - one plain PSUM->SBUF copy, no DMA.
The output is produced as two partial sums (partitions 0-63 even-head part,
64-127 odd) that the host adds and transposes.
"""

import numpy as np

import concourse.bacc as bacc
import concourse.bass as bass
import concourse.mybir as mybir
import concourse.tile as tile
from concourse.bass_utils import run_bass_kernel_spmd

D, T, N, H = 8, 2048, 64, 8
C = 128  # row-chunk size
F16 = mybir.dt.float16
F32 = mybir.dt.float32

EV = [0, 4, 2, 6]  # even-head order in packed layouts
OD = [1, 5, 3, 7]  # odd-head order


def build_nc(t_len: int = T, reps: int = 1, loop_reps: int = 1) -> bacc.Bacc:
    """Build the per-core Bass module.  reps>1 repeats the computation
    python-unrolled; loop_reps>1 wraps it in a hardware For_i loop instead
    (both produce identical output; used for wall-clock delta timing)."""
    nch = t_len // C
    assert nch % 2 == 0
    nc = bacc.Bacc("TRN2", target_bir_lowering=False, debug=False)

    rpt = nc.dram_tensor("rpt", [N, t_len], F16, kind="ExternalInput")
    rp = nc.dram_tensor("rp", [C, nch * N], F16, kind="ExternalInput")
    c_all = nc.dram_tensor("c_all", [2 * N, 4 * t_len], F16, kind="ExternalInput")
    er_all = nc.dram_tensor("er_all", [C, nch * 4 * C], F16, kind="ExternalInput")
    maskd = nc.dram_tensor("maskd", [C, 4 * C], F32, kind="ExternalInput")
    out2 = nc.dram_tensor("out2", [2 * N, t_len], F16, kind="ExternalOutput")

    with tile.TileContext(nc) as tc:
        with (
            tc.tile_pool(name="const", bufs=1) as cpool,
            tc.tile_pool(name="wmp", bufs=2) as wm_pool,
            tc.tile_pool(name="s16p", bufs=2) as s16_pool,
            tc.tile_pool(name="ps_w", bufs=2, space="PSUM") as ps_w,
            tc.tile_pool(name="ps_o", bufs=2, space="PSUM") as ps_o,
            tc.tile_pool(name="ps_s", bufs=1, space="PSUM") as ps_s,
        ):
            # --- constants, loaded once (outside the timed loop) ---
            rpt2_sb = cpool.tile([2 * N, t_len], F16)
            nc.sync.dma_start(rpt2_sb[0:N, :], rpt[:])
            nc.scalar.dma_start(rpt2_sb[N : 2 * N, :], rpt[:])
            rp_sb = cpool.tile([C, nch * N], F16)
            nc.gpsimd.dma_start(rp_sb[:], rp[:])
            c_sb = cpool.tile([2 * N, 4 * t_len], F16)
            for pc in range(4):
                sl = slice(pc * t_len, (pc + 1) * t_len)
                eng = (nc.sync, nc.scalar, nc.gpsimd, nc.sync)[pc]
                eng.dma_start(c_sb[:, sl], c_all[:, sl])
            er_sb = cpool.tile([C, nch * 4 * C], F16)
            for pc in range(4):
                sl = slice(pc * nch * C, (pc + 1) * nch * C)
                eng = (nc.scalar, nc.sync, nc.gpsimd, nc.scalar)[pc]
                eng.dma_start(er_sb[:, sl], er_all[:, sl])
            mask_sb = cpool.tile([C, 4 * C], F32)
            nc.gpsimd.dma_start(mask_sb[:], maskd[:])
            out_sb = cpool.tile([2 * N, t_len], F16)
            # f16 mask copy for the Pool-engine (SBUF-only) mask multiplies
            mask16 = cpool.tile([C, 4 * C], F16)
            nc.vector.tensor_copy(mask16[:], mask_sb[:])

            def one_rep():
                p_s = None

                def emit_w(i):
                    # W_h = A_I C_{h,I}: row-tiled pair, even heads bank A
                    # (operands at partitions 0-63), odd bank B (64-127),
                    # then the tril masks.  Emitted one chunk ahead so the
                    # mask chain overlaps the previous chunk's PE work.
                    tsl = slice(i * C, (i + 1) * C)
                    csl = slice(i * 4 * C, (i + 1) * 4 * C)
                    p_wA = ps_w.tile([C, 4 * C], F32, tag="wA")
                    p_wB = ps_w.tile([C, 4 * C], F32, tag="wB")
                    nc.tensor.matmul(
                        p_wA[:], lhsT=rpt2_sb[0:N, tsl], rhs=c_sb[0:N, csl],
                        start=True, stop=True, tile_position=(0, 0),
                    )
                    nc.tensor.matmul(
                        p_wB[:], lhsT=rpt2_sb[N : 2 * N, tsl],
                        rhs=c_sb[N : 2 * N, csl],
                        start=True, stop=True, tile_position=(64, 0),
                    )
                    # masks: bank A all-DVE; bank B cols 0-255 via
                    # Act copy -> Pool f16 multiply, cols 256-511 DVE
                    wmA = wm_pool.tile([C, 4 * C], F16, tag="wmA")
                    nc.vector.tensor_mul(wmA[:], p_wA[:], mask_sb[:])
                    wmB = wm_pool.tile([C, 4 * C], F16, tag="wmB")
                    w16 = wm_pool.tile([C, 2 * C], F16, tag="w16")
                    nc.scalar.copy(w16[:], p_wB[:, : 2 * C])
                    nc.gpsimd.tensor_mul(
                        wmB[:, : 2 * C], w16[:], mask16[:, : 2 * C]
                    )
                    nc.vector.tensor_mul(
                        wmB[:, 2 * C :], p_wB[:, 2 * C :], mask_sb[:, 2 * C :]
                    )
                    return wmA, wmB

                wm_cur = emit_w(0)
                for i in range(nch):
                    ii, m = i // 2, i % 2
                    esl = i * 4 * C  # er col base, chunk i

                    # state snapshot S_I (pre-update), already stacked:
                    # s16[64e+j, 64g+i2] = S_{(EV|OD)[g]}[j, i2]
                    s16 = None
                    if i > 0:
                        s16 = s16_pool.tile([2 * N, 4 * N], F16, tag="s16")
                        nc.scalar.copy(s16[:], p_s[:])

                    wmA, wmB = wm_cur
                    if i + 1 < nch:
                        wm_cur = emit_w(i + 1)

                    # out^T accumulation: per pair-of-chunks PSUM bank,
                    # partitions 0-63 even-head partial, 64-127 odd partial
                    if m == 0:
                        p_o2 = ps_o.tile([2 * N, 2 * C], F32, tag="o")
                    p_o = p_o2[:, m * C : (m + 1) * C]
                    n_g = 4 if i == 0 else 6  # MMs per col group this chunk
                    g_lo = g_hi = 0
                    # intra: out^T += Er_h^T (mask o W_h), col-tiled pairs
                    for a2 in range(4):
                        nc.tensor.matmul(
                            p_o[0:N, :],
                            lhsT=er_sb[:, esl + 64 * a2 : esl + 64 * (a2 + 1)],
                            rhs=wmA[:, a2 * C : (a2 + 1) * C],
                            start=(g_lo == 0 and m == 0),
                            stop=(g_lo == n_g - 1 and m == 1),
                            skip_group_check=True, tile_position=(0, 0),
                        )
                        g_lo += 1
                        nc.tensor.matmul(
                            p_o[N : 2 * N, :],
                            lhsT=er_sb[:, esl + 4 * N + 64 * a2 : esl + 4 * N + 64 * (a2 + 1)],
                            rhs=wmB[:, a2 * C : (a2 + 1) * C],
                            start=(g_hi == 0 and m == 0),
                            stop=(g_hi == n_g - 1 and m == 1),
                            skip_group_check=True, tile_position=(0, 64),
                        )
                        g_hi += 1
                    # inter: out^T += S_h^T C_h, head pairs stacked on K
                    if i > 0:
                        for a2 in range(4):
                            cg = 0 if a2 < 2 else N
                            nc.tensor.matmul(
                                p_o[cg : cg + N, :],
                                lhsT=s16[:, a2 * N : (a2 + 1) * N],
                                rhs=c_sb[:, i * 4 * C + a2 * C : i * 4 * C + (a2 + 1) * C],
                                start=False,
                                stop=((g_lo if a2 < 2 else g_hi) == n_g - 1 and m == 1),
                                skip_group_check=True, tile_position=(0, cg),
                            )
                            if a2 < 2:
                                g_lo += 1
                            else:
                                g_hi += 1
                    if m == 1:
                        nc.scalar.copy(
                            out_sb[:, ii * 2 * C : (ii + 1) * 2 * C], p_o2[:]
                        )

                    # state update: S_h += A_I^T Er_{h,I}, col-tiled pair
                    # (even heads -> partitions 0-63, odd -> 64-127)
                    if i < nch - 1:
                        if i == 0:
                            p_s = ps_s.tile([2 * N, 4 * N], F32, tag="s")
                        nc.tensor.matmul(
                            p_s[0:N, :],
                            lhsT=rp_sb[:, i * N : (i + 1) * N],
                            rhs=er_sb[:, esl : esl + 4 * N],
                            start=(i == 0), stop=(i == nch - 2),
                            skip_group_check=True, tile_position=(0, 0),
                        )
                        nc.tensor.matmul(
                            p_s[N : 2 * N, :],
                            lhsT=rp_sb[:, i * N : (i + 1) * N],
                            rhs=er_sb[:, esl + 4 * N : esl + 8 * N],
                            start=(i == 0), stop=(i == nch - 2),
                            skip_group_check=True, tile_position=(0, 64),
                        )

                nc.sync.dma_start(out2[:], out_sb[:])

            if loop_reps > 1:
                with tc.For_i(
                    0, loop_reps, 1,
                    hint_engines=(
                        mybir.EngineType.PE,
                        mybir.EngineType.Activation,
                        mybir.EngineType.DVE,
                        mybir.EngineType.Pool,
                        mybir.EngineType.SP,
                    ),
                ):
                    one_rep()
            else:
                for _rep in range(reps):
                    one_rep()

    nc.compile()
    return nc


def _host_prep(r_prime: np.ndarray, Q: np.ndarray, E: np.ndarray, t_len: int = T):
    """Shard + precompute host-side inputs for each of the 8 cores."""
    nch = t_len // C
    order = EV + OD  # head for (e, g) = order[4*e + g]
    mask = np.tile(np.triu(np.ones((C, C), np.float32)), (1, 4))
    # C_h[j, t] = (A Q_h)[t, j];  Er_h[t, i] = (A E_h^T)[t, i]
    A = r_prime  # (D, t_len, N) f32
    AQ = np.einsum("bti,hij->bhtj", A, Q[order], optimize=True)  # (D,8,t,N)
    AE = np.einsum("bti,hji->bhtj", A, E[order], optimize=True)  # (D,8,t,N)
    in_maps = []
    for b in range(D):
        a = A[b]
        rpt16 = np.ascontiguousarray(a.T).astype(np.float16)
        rp16 = (
            a.reshape(nch, C, N).transpose(1, 0, 2).reshape(C, nch * N)
        ).astype(np.float16)
        # c_all[64e+j, 512i + 128g + t'] = C_{order[4e+g]}[j, 128i+t']
        cq = AQ[b].transpose(0, 2, 1).reshape(2, 4, N, nch, C)  # (e,g,j,i,t')
        c16 = np.ascontiguousarray(
            cq.transpose(0, 2, 3, 1, 4).reshape(2 * N, nch * 4 * C)
        ).astype(np.float16)
        # er_all[u, 512i + 256e + 64g + i2] = Er_{order[4e+g]}[128i+u, i2]
        er = AE[b].reshape(2, 4, nch, C, N)  # (e,g,i,u,i2)
        er16 = np.ascontiguousarray(
            er.transpose(3, 2, 0, 1, 4).reshape(C, nch * 4 * C)
        ).astype(np.float16)
        in_maps.append(
            {
                "rpt": rpt16,
                "rp": rp16,
                "c_all": c16,
                "er_all": er16,
                "maskd": mask,
            }
        )
    return in_maps


_NC_CACHE: dict = {}


def kernel(r_prime: np.ndarray, Q: np.ndarray, E: np.ndarray) -> np.ndarray:
    r_prime = np.asarray(r_prime, np.float32)
    Q = np.asarray(Q, np.float32)
    E = np.asarray(E, np.float32)
    t_len = r_prime.shape[1]
    if ("nc", t_len) not in _NC_CACHE:
        _NC_CACHE[("nc", t_len)] = build_nc(t_len)
    nc = _NC_CACHE[("nc", t_len)]
    in_maps = _host_prep(r_prime, Q, E, t_len)
    res = run_bass_kernel_spmd(nc, in_maps, list(range(D)))
    outs = []
    for b in range(D):
        o2 = res.results[b]["out2"].astype(np.float32)  # (128, t_len)
        outs.append(np.ascontiguousarray((o2[0:N] + o2[N : 2 * N]).T))
    return np.stack(outs).astype(np.float32)


# revision 13
# speedup vs baseline: 1.5219x; 1.5219x over previous
"""Trainium2 Bass kernel for nn_AttentionBlock (sparse causal attention).

Math (per batch b, head h), with A = r_prime[b] (T x N):
    Omega_h = tril(A @ Q_h @ A^T)            (T x T)
    out[b]  = sum_h Omega_h @ (A @ E_h^T)    (T x N)

Strategy: data-parallel over batch (8 batches -> 8 NeuronCores).  Host
precomputes the input-linear maps C_h = (A Q_h)^T (N x T) and
Er_h = A E_h^T (T x N) in fp16 (pure GEMM re-layout of the inputs), so the
device only runs the quadratic-in-T part: per row-chunk I (C=128 rows),

    out_I^T = sum_h [ Er_{h,I}^T (mask o W_{h,I})  +  S_{h,I}^T C_{h,I} ]
    W_{h,I} = A_I C_{h,I}   (the diagonal C x C block, tril-masked)
    S_{h,I} = sum_{J<I} A_J^T Er_{h,J}   (N x N running state, PSUM-resident)

All operands live in SBUF for the whole rep (~50 KB/partition).  Heads are
split even/odd across SBUF partition halves so every K=64 matmul runs as a
row-tiled pair (tile_position (0,0)/(64,0), concurrent in the PE array) and
every M=64 matmul as a col-tiled pair ((0,0)/(0,64)).  The state PSUM bank
holds even heads at partitions 0-63 and odd at 64-127, which is exactly the
stacked layout the inter matmul needs - one plain PSUM->SBUF copy, no DMA.
The output is produced as two partial sums (partitions 0-63 even-head part,
64-127 odd) that the host adds and transposes.
"""

import numpy as np

import concourse.bacc as bacc
import concourse.bass as bass
import concourse.mybir as mybir
import concourse.tile as tile
from concourse.bass_utils import run_bass_kernel_spmd

D, T, N, H = 8, 2048, 64, 8
C = 128  # row-chunk size
F16 = mybir.dt.float16
F32 = mybir.dt.float32

EV = [0, 4, 2, 6]  # even-head order in packed layouts
OD = [1, 5, 3, 7]  # odd-head order


def build_nc(t_len: int = T, reps: int = 1, loop_reps: int = 1) -> bacc.Bacc:
    """Build the per-core Bass module.  reps>1 repeats the computation
    python-unrolled; loop_reps>1 wraps it in a hardware For_i loop instead
    (both produce identical output; used for wall-clock delta timing)."""
    nch = t_len // C
    assert nch % 2 == 0
    nc = bacc.Bacc("TRN2", target_bir_lowering=False, debug=False)

    rpt = nc.dram_tensor("rpt", [N, t_len], F16, kind="ExternalInput")
    rp = nc.dram_tensor("rp", [C, nch * N], F16, kind="ExternalInput")
    c_all = nc.dram_tensor("c_all", [2 * N, 4 * t_len], F16, kind="ExternalInput")
    er_all = nc.dram_tensor("er_all", [C, nch * 4 * C], F16, kind="ExternalInput")
    maskd = nc.dram_tensor("maskd", [C, 4 * C], F32, kind="ExternalInput")
    out2 = nc.dram_tensor("out2", [2 * N, t_len], F16, kind="ExternalOutput")

    with tile.TileContext(nc) as tc:
        with (
            tc.tile_pool(name="const", bufs=1) as cpool,
            tc.tile_pool(name="wmp", bufs=3) as wm_pool,
            tc.tile_pool(name="s16p", bufs=2) as s16_pool,
            tc.tile_pool(name="ps_w", bufs=3, space="PSUM") as ps_w,
            tc.tile_pool(name="ps_o", bufs=1, space="PSUM") as ps_o,
            tc.tile_pool(name="ps_s", bufs=1, space="PSUM") as ps_s,
        ):
            # --- constants, loaded once (outside the timed loop) ---
            rpt2_sb = cpool.tile([2 * N, t_len], F16)
            nc.sync.dma_start(rpt2_sb[0:N, :], rpt[:])
            nc.scalar.dma_start(rpt2_sb[N : 2 * N, :], rpt[:])
            rp_sb = cpool.tile([C, nch * N], F16)
            nc.gpsimd.dma_start(rp_sb[:], rp[:])
            c_sb = cpool.tile([2 * N, 4 * t_len], F16)
            for pc in range(4):
                sl = slice(pc * t_len, (pc + 1) * t_len)
                eng = (nc.sync, nc.scalar, nc.gpsimd, nc.sync)[pc]
                eng.dma_start(c_sb[:, sl], c_all[:, sl])
            er_sb = cpool.tile([C, nch * 4 * C], F16)
            for pc in range(4):
                sl = slice(pc * nch * C, (pc + 1) * nch * C)
                eng = (nc.scalar, nc.sync, nc.gpsimd, nc.scalar)[pc]
                eng.dma_start(er_sb[:, sl], er_all[:, sl])
            mask_sb = cpool.tile([C, 4 * C], F32)
            nc.gpsimd.dma_start(mask_sb[:], maskd[:])
            out_sb = cpool.tile([2 * N, t_len], F16)
            # f16 mask copy for the Pool-engine (SBUF-only) mask multiplies
            mask16 = cpool.tile([C, 4 * C], F16)
            nc.vector.tensor_copy(mask16[:], mask_sb[:])

            def one_rep():
                p_s = None

                def emit_w(i):
                    # W_h = A_I C_{h,I}: row-tiled pair, even heads bank A
                    # (operands at partitions 0-63), odd bank B (64-127),
                    # then the tril masks.  Emitted one chunk ahead so the
                    # mask chain overlaps the previous chunk's PE work.
                    tsl = slice(i * C, (i + 1) * C)
                    csl = slice(i * 4 * C, (i + 1) * 4 * C)
                    p_wA = ps_w.tile([C, 4 * C], F32, tag="wA")
                    p_wB = ps_w.tile([C, 4 * C], F32, tag="wB")
                    nc.tensor.matmul(
                        p_wA[:], lhsT=rpt2_sb[0:N, tsl], rhs=c_sb[0:N, csl],
                        start=True, stop=True, tile_position=(0, 0),
                    )
                    nc.tensor.matmul(
                        p_wB[:], lhsT=rpt2_sb[N : 2 * N, tsl],
                        rhs=c_sb[N : 2 * N, csl],
                        start=True, stop=True, tile_position=(64, 0),
                    )
                    # masks: bank A all-DVE; bank B cols 0-255 via
                    # Act copy -> Pool f16 multiply, cols 256-511 DVE
                    wmA = wm_pool.tile([C, 4 * C], F16, tag="wmA")
                    nc.vector.tensor_mul(wmA[:], p_wA[:], mask_sb[:])
                    wmB = wm_pool.tile([C, 4 * C], F16, tag="wmB")
                    w16 = wm_pool.tile([C, 2 * C], F16, tag="w16")
                    nc.scalar.copy(w16[:], p_wB[:, : 2 * C])
                    nc.gpsimd.tensor_mul(
                        wmB[:, : 2 * C], w16[:], mask16[:, : 2 * C]
                    )
                    nc.vector.tensor_mul(
                        wmB[:, 2 * C :], p_wB[:, 2 * C :], mask_sb[:, 2 * C :]
                    )
                    return wmA, wmB

                wm_q = [emit_w(0), emit_w(1)]
                for i in range(nch):
                    ii, m = i // 2, i % 2
                    esl = i * 4 * C  # er col base, chunk i

                    # state snapshot S_I (pre-update), already stacked:
                    # s16[64e+j, 64g+i2] = S_{(EV|OD)[g]}[j, i2]
                    s16 = None
                    if i > 0:
                        s16 = s16_pool.tile([2 * N, 4 * N], F16, tag="s16")
                        nc.scalar.copy(s16[:], p_s[:])

                    wmA, wmB = wm_q.pop(0)
                    if i + 2 < nch:
                        wm_q.append(emit_w(i + 2))

                    # out^T accumulation: per pair-of-chunks PSUM bank,
                    # partitions 0-63 even-head partial, 64-127 odd partial
                    if m == 0:
                        p_o2 = ps_o.tile([2 * N, 2 * C], F32, tag="o")
                    p_o = p_o2[:, m * C : (m + 1) * C]
                    n_g = 4 if i == 0 else 6  # MMs per col group this chunk
                    g_lo = g_hi = 0
                    # intra: out^T += Er_h^T (mask o W_h), col-tiled pairs
                    for a2 in range(4):
                        nc.tensor.matmul(
                            p_o[0:N, :],
                            lhsT=er_sb[:, esl + 64 * a2 : esl + 64 * (a2 + 1)],
                            rhs=wmA[:, a2 * C : (a2 + 1) * C],
                            start=(g_lo == 0 and m == 0),
                            stop=(g_lo == n_g - 1 and m == 1),
                            skip_group_check=True, tile_position=(0, 0),
                        )
                        g_lo += 1
                        nc.tensor.matmul(
                            p_o[N : 2 * N, :],
                            lhsT=er_sb[:, esl + 4 * N + 64 * a2 : esl + 4 * N + 64 * (a2 + 1)],
                            rhs=wmB[:, a2 * C : (a2 + 1) * C],
                            start=(g_hi == 0 and m == 0),
                            stop=(g_hi == n_g - 1 and m == 1),
                            skip_group_check=True, tile_position=(0, 64),
                        )
                        g_hi += 1
                    # inter: out^T += S_h^T C_h, head pairs stacked on K
                    if i > 0:
                        for a2 in range(4):
                            cg = 0 if a2 < 2 else N
                            nc.tensor.matmul(
                                p_o[cg : cg + N, :],
                                lhsT=s16[:, a2 * N : (a2 + 1) * N],
                                rhs=c_sb[:, i * 4 * C + a2 * C : i * 4 * C + (a2 + 1) * C],
                                start=False,
                                stop=((g_lo if a2 < 2 else g_hi) == n_g - 1 and m == 1),
                                skip_group_check=True, tile_position=(0, cg),
                            )
                            if a2 < 2:
                                g_lo += 1
                            else:
                                g_hi += 1
                    if m == 1:
                        nc.scalar.copy(
                            out_sb[:, ii * 2 * C : (ii + 1) * 2 * C], p_o2[:]
                        )

                    # state update: S_h += A_I^T Er_{h,I}, col-tiled pair
                    # (even heads -> partitions 0-63, odd -> 64-127)
                    if i < nch - 1:
                        if i == 0:
                            p_s = ps_s.tile([2 * N, 4 * N], F32, tag="s")
                        nc.tensor.matmul(
                            p_s[0:N, :],
                            lhsT=rp_sb[:, i * N : (i + 1) * N],
                            rhs=er_sb[:, esl : esl + 4 * N],
                            start=(i == 0), stop=(i == nch - 2),
                            skip_group_check=True, tile_position=(0, 0),
                        )
                        nc.tensor.matmul(
                            p_s[N : 2 * N, :],
                            lhsT=rp_sb[:, i * N : (i + 1) * N],
                            rhs=er_sb[:, esl + 4 * N : esl + 8 * N],
                            start=(i == 0), stop=(i == nch - 2),
                            skip_group_check=True, tile_position=(0, 64),
                        )

                nc.sync.dma_start(out2[:], out_sb[:])

            if loop_reps > 1:
                with tc.For_i(
                    0, loop_reps, 1,
                    hint_engines=(
                        mybir.EngineType.PE,
                        mybir.EngineType.Activation,
                        mybir.EngineType.DVE,
                        mybir.EngineType.Pool,
                        mybir.EngineType.SP,
                    ),
                ):
                    one_rep()
            else:
                for _rep in range(reps):
                    one_rep()

    nc.compile()
    return nc


def _host_prep(r_prime: np.ndarray, Q: np.ndarray, E: np.ndarray, t_len: int = T):
    """Shard + precompute host-side inputs for each of the 8 cores."""
    nch = t_len // C
    order = EV + OD  # head for (e, g) = order[4*e + g]
    mask = np.tile(np.triu(np.ones((C, C), np.float32)), (1, 4))
    # C_h[j, t] = (A Q_h)[t, j];  Er_h[t, i] = (A E_h^T)[t, i]
    A = r_prime  # (D, t_len, N) f32
    AQ = np.einsum("bti,hij->bhtj", A, Q[order], optimize=True)  # (D,8,t,N)
    AE = np.einsum("bti,hji->bhtj", A, E[order], optimize=True)  # (D,8,t,N)
    in_maps = []
    for b in range(D):
        a = A[b]
        rpt16 = np.ascontiguousarray(a.T).astype(np.float16)
        rp16 = (
            a.reshape(nch, C, N).transpose(1, 0, 2).reshape(C, nch * N)
        ).astype(np.float16)
        # c_all[64e+j, 512i + 128g + t'] = C_{order[4e+g]}[j, 128i+t']
        cq = AQ[b].transpose(0, 2, 1).reshape(2, 4, N, nch, C)  # (e,g,j,i,t')
        c16 = np.ascontiguousarray(
            cq.transpose(0, 2, 3, 1, 4).reshape(2 * N, nch * 4 * C)
        ).astype(np.float16)
        # er_all[u, 512i + 256e + 64g + i2] = Er_{order[4e+g]}[128i+u, i2]
        er = AE[b].reshape(2, 4, nch, C, N)  # (e,g,i,u,i2)
        er16 = np.ascontiguousarray(
            er.transpose(3, 2, 0, 1, 4).reshape(C, nch * 4 * C)
        ).astype(np.float16)
        in_maps.append(
            {
                "rpt": rpt16,
                "rp": rp16,
                "c_all": c16,
                "er_all": er16,
                "maskd": mask,
            }
        )
    return in_maps


_NC_CACHE: dict = {}


def kernel(r_prime: np.ndarray, Q: np.ndarray, E: np.ndarray) -> np.ndarray:
    r_prime = np.asarray(r_prime, np.float32)
    Q = np.asarray(Q, np.float32)
    E = np.asarray(E, np.float32)
    t_len = r_prime.shape[1]
    if ("nc", t_len) not in _NC_CACHE:
        _NC_CACHE[("nc", t_len)] = build_nc(t_len)
    nc = _NC_CACHE[("nc", t_len)]
    in_maps = _host_prep(r_prime, Q, E, t_len)
    res = run_bass_kernel_spmd(nc, in_maps, list(range(D)))
    outs = []
    for b in range(D):
        o2 = res.results[b]["out2"].astype(np.float32)  # (128, t_len)
        outs.append(np.ascontiguousarray((o2[0:N] + o2[N : 2 * N]).T))
    return np.stack(outs).astype(np.float32)
